# revision 8
# baseline (speedup 1.0000x reference)
"""DisplaceChannel (integer displace + per-position 5x5 gaussian depthwise
conv) as a Bass/Tile kernel for 8 Trainium2 NeuronCores — v6.

Math: separable gaussian; integer shift + 'same' zero-padding fold into
banded 64x64 row/col operators built host-side.  Per image:
out = R1^T X R2.

Per 2-group block (2 positions x 8 channels x one batch-pair), PE work:
  pass1 (image-stationary, transposing): 8x MM N=128, lhsT = wt[:, 128]
        (2 channels, both batches), rhs = blockdiag(R1,R1)
  pass2 (operator-stationary): 2x MM N=512, lhsT = blockdiag(R2,R2),
        rhs = pass1 result (fused fp16 copy [128,1024])
PSUM evacuation is fused across the 2 groups ([128,1024] copies) to
amortize per-op overhead on Vector/Scalar.

DMA rings: input on sync HWDGE (pure stream, nothing blocking it),
operator consts on scalar HWDGE (parallel FIFO), outputs on the
otherwise-idle GpSimd SWDGE ring.  Input chunks ramp 2->12 groups.

Sharding: data-parallel over batch (4 per core); operators replicated.
"""

import numpy as np

from concourse import bacc, mybir, tile
from concourse.bass_utils import run_bass_kernel_spmd

# problem constants (hardcoded per harness contract)
B_FULL, C, H, W = 32, 384, 64, 64
N_CORES = 8
B_LOC = B_FULL // N_CORES          # 4 batches per core
P_POS = 48                         # offset positions; C // P_POS = 8 chan/pos
GROUP = C // P_POS                 # 8 channels share one operator pair
KSZ, SIGMA, CK = 5, 0.5, 2

N_BPAIR = B_LOC // 2               # batch-pairs (2bp, 2bp+1) per core
XCOLS = C * 64                     # 24576 per-bp packed cols
GCOLS = GROUP * 64                 # 512 cols per group

FP16 = mybir.dt.float16
FP32 = mybir.dt.float32

_LAST_RESULT = None                # test.py introspection (profile/exec time)

# per-bp input chunk schedule (group-start offsets), output chunks, and
# operator-constant slices (position ranges)
IN_STARTS = {0: [0, 2, 4, 8, 12, 24, 36], 1: [0, 12, 24, 36]}
OUT_CHUNKS = [(0, 6), (6, 12), (12, 18), (18, 24), (24, 30), (30, 36),
              (36, 42), (42, 46), (46, 48)]
OPS_SLICES = [(0, 2), (2, 24), (24, 48)]


def _shift_conv_matrix(sub, d):
    """[64(src), 64(out)] with R[src,out] = k[i], src = out + i - 2 - d,
    masked by conv zero-pad (0<=out+i-2<64) and shift zero-fill (0<=src<64)."""
    k = np.exp(-((np.arange(KSZ) - CK + sub) ** 2) / (2.0 * SIGMA**2))
    k = k / k.sum()
    R = np.zeros((H, H), dtype=np.float64)
    out = np.arange(H)
    for i in range(KSZ):
        t = out + i - CK            # coordinate in the shifted image
        src = t - d
        m = (t >= 0) & (t < H) & (src >= 0) & (src < H)
        R[src[m], out[m]] += k[i]
    return R


def _build_ops(offset):
    """ops1 [128, 48*128] fp16 : per position blockdiag(R1, R1)
    ops2 [128, 48*128] fp16 : per position blockdiag(R2, R2)"""
    off_round = np.round(offset.astype(np.float64))
    off_int = off_round.astype(np.int64)
    sub = offset.astype(np.float64) - off_round
    ops1 = np.zeros((128, P_POS * 128), dtype=np.float64)
    ops2 = np.zeros((128, P_POS * 128), dtype=np.float64)
    for p in range(P_POS):
        R1 = _shift_conv_matrix(sub[p, 1], off_int[p, 1])   # y: suby, dy
        R2 = _shift_conv_matrix(sub[p, 0], off_int[p, 0])   # x: subx, dx
        ops1[0:64, 128 * p:128 * p + 64] = R1
        ops1[64:128, 128 * p + 64:128 * p + 128] = R1
        ops2[0:64, 128 * p:128 * p + 64] = R2
        ops2[64:128, 128 * p + 64:128 * p + 128] = R2
    return ops1.astype(np.float16), ops2.astype(np.float16)


def _build_bass():
    nc = bacc.Bacc(
        "TRN2",
        target_bir_lowering=False,
        debug=False,
        num_devices=N_CORES,
    )
    # packed fp16 input: per bp a [128, 24576] block; channel c at cols
    # 64c:64c+64, batch 2bp rows on partitions 0:64, batch 2bp+1 on 64:128.
    x_in = nc.declare_dram_parameter("x", [N_BPAIR, 128, XCOLS], FP16,
                                     isOutput=False)
    ops1_in = nc.declare_dram_parameter("ops1", [128, P_POS * 128], FP16,
                                        isOutput=False)
    ops2_in = nc.declare_dram_parameter("ops2", [128, P_POS * 128], FP16,
                                        isOutput=False)
    # packed output: per bp [128, 24576] fp16 (host upcasts to f32);
    # partitions (c2, x'), cols (g, m, s, y'); channel = 8g + 2m + c2,
    # batch = 2bp + s.
    y_out = nc.declare_dram_parameter("y", [N_BPAIR, 128, XCOLS], FP16,
                                      isOutput=True)

    with tile.TileContext(nc) as tc:
        with (
            tc.tile_pool(name="consts", bufs=1) as consts,
            tc.tile_pool(name="wsmall", bufs=2) as wpool,
            tc.tile_pool(name="w12", bufs=6) as wpool12,
            tc.tile_pool(name="l2", bufs=4) as l2pool,
            tc.tile_pool(name="outs", bufs=4) as outpool,
            tc.tile_pool(name="psum1", bufs=3, space="PSUM") as psum1p,
            tc.tile_pool(name="psum2", bufs=3, space="PSUM") as psum2p,
        ):
            # operator constants, sliced so early groups unblock fast;
            # first (tiny) slice on the sync ring ahead of the input
            # stream, the rest on the scalar HWDGE ring in parallel.
            t_ops1, t_ops2 = {}, {}
            for (a, b) in OPS_SLICES:
                t_ops1[a] = consts.tile([128, (b - a) * 128], FP16,
                                        tag=f"ops1_{a}", name=f"t_ops1_{a}")
                t_ops2[a] = consts.tile([128, (b - a) * 128], FP16,
                                        tag=f"ops2_{a}", name=f"t_ops2_{a}")

            def ops_slice(t, g):
                for (a, b) in OPS_SLICES:
                    if a <= g < b:
                        return t[a][:, 128 * (g - a):128 * (g - a) + 128]
                raise AssertionError(g)

            a0, b0 = OPS_SLICES[0]
            nc.sync.dma_start(out=t_ops1[a0][:],
                              in_=ops1_in[:, 128 * a0:128 * b0])

            # input chunk bookkeeping (sync HWDGE ring, emission order =
            # trigger order; pool bufs throttle prefetch depth)
            chunk_list = []                     # (bp, ga, gb)
            for bp in range(N_BPAIR):
                starts = IN_STARTS[bp]
                for ci, a in enumerate(starts):
                    b = starts[ci + 1] if ci + 1 < len(starts) else P_POS
                    chunk_list.append((bp, a, b))
            g2chunk = {}
            for ci, (bp, a, b) in enumerate(chunk_list):
                for g in range(a, b):
                    g2chunk[(bp, g)] = ci
            wts = {}
            n_emitted = 0

            def emit_in_dma(ci):
                bp, a, b = chunk_list[ci]
                gsz = b - a
                pool = wpool12 if gsz == 12 else wpool
                wt = pool.tile([128, gsz * GCOLS], FP16, tag=f"w{gsz}")
                nc.sync.dma_start(
                    out=wt[:],
                    in_=x_in[bp][:, a * GCOLS:b * GCOLS])
                wts[ci] = wt

            # ramp order on the sync ring: ops1 head, first input chunk,
            # ops2 head, second chunk; const tails on the scalar ring.
            emit_in_dma(0)
            nc.sync.dma_start(out=t_ops2[a0][:],
                              in_=ops2_in[:, 128 * a0:128 * b0])
            emit_in_dma(1)
            n_emitted = 2
            for (a, b) in OPS_SLICES[1:]:
                nc.scalar.dma_start(out=t_ops1[a][:],
                                    in_=ops1_in[:, 128 * a:128 * b])
                nc.scalar.dma_start(out=t_ops2[a][:],
                                    in_=ops2_in[:, 128 * a:128 * b])

            LOOKAHEAD = 4
            it = 0
            outs = None
            oc = None
            for bp in range(N_BPAIR):
                for g in range(P_POS):                  # 48 groups / bp
                    ci = g2chunk[(bp, g)]
                    while n_emitted <= min(ci + LOOKAHEAD, len(chunk_list) - 1):
                        emit_in_dma(n_emitted)
                        n_emitted += 1
                    for s_, e_ in OUT_CHUNKS:
                        if g == s_:
                            outs = outpool.tile([128, (e_ - s_) * GCOLS],
                                                FP16, tag="outs")
                            oc = (s_, e_)
                    wt = wts[ci]
                    ca = chunk_list[ci][1]          # chunk's first group

                    ps1 = psum1p.tile([128, 512], FP32)
                    r1 = ops_slice(t_ops1, g)
                    for m in range(GROUP // 2):          # 4 channel-pairs
                        cs = slice((g - ca) * GCOLS + 128 * m,
                                   (g - ca) * GCOLS + 128 * m + 128)
                        nc.tensor.matmul(ps1[:, 128 * m:128 * m + 128],
                                         wt[:, cs], r1,
                                         start=True, stop=True)
                    # psum fp32 -> sbuf fp16 (pass2 moving operand)
                    l2 = l2pool.tile([128, 512], FP16)
                    if it % 2 == 0:
                        nc.vector.tensor_copy(l2[:], ps1[:])
                    else:
                        nc.scalar.copy(l2[:], ps1[:])
                    ps2 = psum2p.tile([128, 512], FP32)
                    nc.tensor.matmul(ps2[:], ops_slice(t_ops2, g), l2[:],
                                     start=True, stop=True)
                    # final psum -> staging (fp16)
                    od = outs[:, 512 * (g - oc[0]):512 * (g - oc[0]) + 512]
                    if it % 2 == 0:
                        nc.scalar.copy(od, ps2[:])
                    else:
                        nc.vector.tensor_copy(od, ps2[:])
                    it += 1

                    if g + 1 == oc[1]:
                        nc.gpsimd.dma_start(
                            out=y_out[bp][:, GCOLS * oc[0]:GCOLS * oc[1]],
                            in_=outs[:])
    nc.compile()
    return nc


_NC_CACHE = None


def kernel(x: np.ndarray, offset: np.ndarray) -> np.ndarray:
    global _LAST_RESULT, _NC_CACHE
    assert x.shape == (B_FULL, C, H, W), x.shape
    ops1, ops2 = _build_ops(np.asarray(offset, dtype=np.float32))
    if _NC_CACHE is None:
        _NC_CACHE = _build_bass()
    nc = _NC_CACHE

    # host pack: fp16 cast + [p, (c, x)] layout; batch 2bp rows on
    # partitions 0:64, batch 2bp+1 rows on 64:128 (index permutation only).
    x16 = np.asarray(x, dtype=np.float32).astype(np.float16)
    xv = x16.reshape(N_CORES, N_BPAIR, 2, C, H, W)
    xP = np.empty((N_CORES, N_BPAIR, 128, C, W), dtype=np.float16)
    xP[:, :, 0:64] = xv[:, :, 0].transpose(0, 1, 3, 2, 4)   # [i,bp,y,c,x]
    xP[:, :, 64:128] = xv[:, :, 1].transpose(0, 1, 3, 2, 4)
    xP = xP.reshape(N_CORES, N_BPAIR, 128, XCOLS)

    in_maps = []
    for i in range(N_CORES):
        in_maps.append({"x": xP[i], "ops1": ops1, "ops2": ops2})
    res = run_bass_kernel_spmd(nc, in_maps, list(range(N_CORES)))
    _LAST_RESULT = res

    # host unpack: y[i] [bp, (c2, x'), (g, m, s, y')];
    # channel = 8g + 2m + c2, batch = 4i + 2bp + s.
    out = np.empty((B_FULL, C, H, W), dtype=np.float32)
    for i in range(N_CORES):
        yv = res.results[i]["y"].astype(np.float32).reshape(
            N_BPAIR, 2, W, P_POS, GROUP // 2, 2, H)
        yt = yv.transpose(0, 5, 3, 4, 1, 6, 2)   # bp s g m c2 y' x'
        out[4 * i:4 * i + 4] = yt.reshape(B_LOC, C, H, W)
    return out


if __name__ == "__main__":
    nc = _build_bass()
    print("bass program built ok")


# revision 9
# speedup vs baseline: 1.0490x; 1.0490x over previous
"""DisplaceChannel (integer displace + per-position 5x5 gaussian depthwise
conv) as a Bass/Tile kernel for 8 Trainium2 NeuronCores — v7.

Math: separable gaussian; integer shift + 'same' zero-padding fold into
banded 64x64 row/col operators built host-side.  Per image:
out = R1^T X R2.

Per 2-group block (2 positions x 8 channels x one batch-pair), PE work:
  pass1 (image-stationary, transposing): 8x MM N=128, lhsT = wt[:, 128]
        (2 channels, both batches), rhs = blockdiag(R1,R1)
  pass2 (operator-stationary): 2x MM N=512, lhsT = blockdiag(R2,R2),
        rhs = pass1 result (fused fp16 copy [128,1024])
PSUM evacuation is fused across the 2 groups ([128,1024] copies) to
amortize per-op overhead on Vector/Scalar.

DMA rings: input on sync HWDGE (pure stream, nothing blocking it),
operator consts on scalar HWDGE (parallel FIFO), outputs on the
otherwise-idle GpSimd SWDGE ring.  Input chunks ramp 2->12 groups.

Sharding: data-parallel over batch (4 per core); operators replicated.
"""

import numpy as np

from concourse import bacc, mybir, tile
from concourse.bass_utils import run_bass_kernel_spmd

# problem constants (hardcoded per harness contract)
B_FULL, C, H, W = 32, 384, 64, 64
N_CORES = 8
B_LOC = B_FULL // N_CORES          # 4 batches per core
P_POS = 48                         # offset positions; C // P_POS = 8 chan/pos
GROUP = C // P_POS                 # 8 channels share one operator pair
KSZ, SIGMA, CK = 5, 0.5, 2

N_BPAIR = B_LOC // 2               # batch-pairs (2bp, 2bp+1) per core
XCOLS = C * 64                     # 24576 per-bp packed cols
GCOLS = GROUP * 64                 # 512 cols per group

FP16 = mybir.dt.float16
FP32 = mybir.dt.float32

_LAST_RESULT = None                # test.py introspection (profile/exec time)

# per-bp input chunk schedule (group-start offsets), output chunks, and
# operator-constant slices (position ranges)
IN_STARTS = {0: [0, 2, 4, 8, 12, 24, 36], 1: [0, 12, 24, 36]}
OUT_CHUNKS = [(0, 4), (4, 12), (12, 24), (24, 36), (36, 44), (44, 48)]
OPS_SLICES = [(0, 2), (2, 24), (24, 48)]
GPSIMD_IN_CHUNKS = (4, 5)      # bp0 [12:24], [24:36] pre-issued on SWDGE


def _shift_conv_matrix(sub, d):
    """[64(src), 64(out)] with R[src,out] = k[i], src = out + i - 2 - d,
    masked by conv zero-pad (0<=out+i-2<64) and shift zero-fill (0<=src<64)."""
    k = np.exp(-((np.arange(KSZ) - CK + sub) ** 2) / (2.0 * SIGMA**2))
    k = k / k.sum()
    R = np.zeros((H, H), dtype=np.float64)
    out = np.arange(H)
    for i in range(KSZ):
        t = out + i - CK            # coordinate in the shifted image
        src = t - d
        m = (t >= 0) & (t < H) & (src >= 0) & (src < H)
        R[src[m], out[m]] += k[i]
    return R


def _build_ops(offset):
    """ops1 [128, 48*128] fp16 : per position blockdiag(R1, R1)
    ops2 [128, 48*128] fp16 : per position blockdiag(R2, R2)"""
    off_round = np.round(offset.astype(np.float64))
    off_int = off_round.astype(np.int64)
    sub = offset.astype(np.float64) - off_round
    ops1 = np.zeros((128, P_POS * 128), dtype=np.float64)
    ops2 = np.zeros((128, P_POS * 128), dtype=np.float64)
    for p in range(P_POS):
        R1 = _shift_conv_matrix(sub[p, 1], off_int[p, 1])   # y: suby, dy
        R2 = _shift_conv_matrix(sub[p, 0], off_int[p, 0])   # x: subx, dx
        ops1[0:64, 128 * p:128 * p + 64] = R1
        ops1[64:128, 128 * p + 64:128 * p + 128] = R1
        ops2[0:64, 128 * p:128 * p + 64] = R2
        ops2[64:128, 128 * p + 64:128 * p + 128] = R2
    return ops1.astype(np.float16), ops2.astype(np.float16)


def _build_bass():
    nc = bacc.Bacc(
        "TRN2",
        target_bir_lowering=False,
        debug=False,
        num_devices=N_CORES,
    )
    # packed fp16 input: per bp a [128, 24576] block; channel c at cols
    # 64c:64c+64, batch 2bp rows on partitions 0:64, batch 2bp+1 on 64:128.
    x_in = nc.declare_dram_parameter("x", [N_BPAIR, 128, XCOLS], FP16,
                                     isOutput=False)
    ops1_in = nc.declare_dram_parameter("ops1", [128, P_POS * 128], FP16,
                                        isOutput=False)
    ops2_in = nc.declare_dram_parameter("ops2", [128, P_POS * 128], FP16,
                                        isOutput=False)
    # packed output: per bp [128, 24576] fp16 (host upcasts to f32);
    # partitions (c2, x'), cols (g, m, s, y'); channel = 8g + 2m + c2,
    # batch = 2bp + s.
    y_out = nc.declare_dram_parameter("y", [N_BPAIR, 128, XCOLS], FP16,
                                      isOutput=True)

    with tile.TileContext(nc) as tc:
        with (
            tc.tile_pool(name="consts", bufs=1) as consts,
            tc.tile_pool(name="wsmall", bufs=2) as wpool,
            tc.tile_pool(name="w12", bufs=6) as wpool12,
            tc.tile_pool(name="l2", bufs=4) as l2pool,
            tc.tile_pool(name="outs", bufs=5) as outpool,
            tc.tile_pool(name="psum1", bufs=3, space="PSUM") as psum1p,
            tc.tile_pool(name="psum2", bufs=3, space="PSUM") as psum2p,
        ):
            # operator constants, sliced so early groups unblock fast;
            # first (tiny) slice on the sync ring ahead of the input
            # stream, the rest on the scalar HWDGE ring in parallel.
            t_ops1, t_ops2 = {}, {}
            for (a, b) in OPS_SLICES:
                t_ops1[a] = consts.tile([128, (b - a) * 128], FP16,
                                        tag=f"ops1_{a}", name=f"t_ops1_{a}")
                t_ops2[a] = consts.tile([128, (b - a) * 128], FP16,
                                        tag=f"ops2_{a}", name=f"t_ops2_{a}")

            def ops_slice(t, g):
                for (a, b) in OPS_SLICES:
                    if a <= g < b:
                        return t[a][:, 128 * (g - a):128 * (g - a) + 128]
                raise AssertionError(g)

            a0, b0 = OPS_SLICES[0]
            nc.sync.dma_start(out=t_ops1[a0][:],
                              in_=ops1_in[:, 128 * a0:128 * b0])

            # input chunk bookkeeping (sync HWDGE ring, emission order =
            # trigger order; pool bufs throttle prefetch depth)
            chunk_list = []                     # (bp, ga, gb)
            for bp in range(N_BPAIR):
                starts = IN_STARTS[bp]
                for ci, a in enumerate(starts):
                    b = starts[ci + 1] if ci + 1 < len(starts) else P_POS
                    chunk_list.append((bp, a, b))
            g2chunk = {}
            for ci, (bp, a, b) in enumerate(chunk_list):
                for g in range(a, b):
                    g2chunk[(bp, g)] = ci
            wts = {}
            n_emitted = 0

            def emit_in_dma(ci, ring=None):
                if ci in wts:
                    return
                bp, a, b = chunk_list[ci]
                gsz = b - a
                pool = wpool12 if gsz == 12 else wpool
                wt = pool.tile([128, gsz * GCOLS], FP16, tag=f"w{gsz}")
                (ring or nc.sync).dma_start(
                    out=wt[:],
                    in_=x_in[bp][:, a * GCOLS:b * GCOLS])
                wts[ci] = wt

            # ramp order on the sync ring: ops1 head, first input chunk,
            # ops2 head, second chunk.  Two bp0 mid chunks go out on the
            # (otherwise output-only) SWDGE ring immediately - its slots
            # are guaranteed free here, so nothing blocks the out triggers
            # queued behind them - doubling early input bandwidth.  Const
            # tails stream on the scalar HWDGE ring in parallel.
            emit_in_dma(0)
            nc.sync.dma_start(out=t_ops2[a0][:],
                              in_=ops2_in[:, 128 * a0:128 * b0])
            emit_in_dma(1)
            for ci in GPSIMD_IN_CHUNKS:
                emit_in_dma(ci, ring=nc.gpsimd)
            for (a, b) in OPS_SLICES[1:]:
                nc.scalar.dma_start(out=t_ops1[a][:],
                                    in_=ops1_in[:, 128 * a:128 * b])
                nc.scalar.dma_start(out=t_ops2[a][:],
                                    in_=ops2_in[:, 128 * a:128 * b])

            LOOKAHEAD = 3
            it = 0
            outs = None
            oc = None
            for bp in range(N_BPAIR):
                for g in range(P_POS):                  # 48 groups / bp
                    ci = g2chunk[(bp, g)]
                    while n_emitted <= min(ci + LOOKAHEAD, len(chunk_list) - 1):
                        emit_in_dma(n_emitted)
                        n_emitted += 1
                    for s_, e_ in OUT_CHUNKS:
                        if g == s_:
                            outs = outpool.tile([128, (e_ - s_) * GCOLS],
                                                FP16, tag="outs")
                            oc = (s_, e_)
                    wt = wts[ci]
                    ca = chunk_list[ci][1]          # chunk's first group

                    ps1 = psum1p.tile([128, 512], FP32)
                    r1 = ops_slice(t_ops1, g)
                    for m in range(GROUP // 2):          # 4 channel-pairs
                        cs = slice((g - ca) * GCOLS + 128 * m,
                                   (g - ca) * GCOLS + 128 * m + 128)
                        nc.tensor.matmul(ps1[:, 128 * m:128 * m + 128],
                                         wt[:, cs], r1,
                                         start=True, stop=True)
                    # psum fp32 -> sbuf fp16 (pass2 moving operand)
                    l2 = l2pool.tile([128, 512], FP16)
                    if it % 2 == 0:
                        nc.vector.tensor_copy(l2[:], ps1[:])
                    else:
                        nc.scalar.copy(l2[:], ps1[:])
                    ps2 = psum2p.tile([128, 512], FP32)
                    nc.tensor.matmul(ps2[:], ops_slice(t_ops2, g), l2[:],
                                     start=True, stop=True)
                    # final psum -> staging (fp16)
                    od = outs[:, 512 * (g - oc[0]):512 * (g - oc[0]) + 512]
                    if it % 2 == 0:
                        nc.scalar.copy(od, ps2[:])
                    else:
                        nc.vector.tensor_copy(od, ps2[:])
                    it += 1

                    if g + 1 == oc[1]:
                        nc.gpsimd.dma_start(
                            out=y_out[bp][:, GCOLS * oc[0]:GCOLS * oc[1]],
                            in_=outs[:])
    nc.compile()
    return nc


_NC_CACHE = None


def kernel(x: np.ndarray, offset: np.ndarray) -> np.ndarray:
    global _LAST_RESULT, _NC_CACHE
    assert x.shape == (B_FULL, C, H, W), x.shape
    ops1, ops2 = _build_ops(np.asarray(offset, dtype=np.float32))
    if _NC_CACHE is None:
        _NC_CACHE = _build_bass()
    nc = _NC_CACHE

    # host pack: fp16 cast + [p, (c, x)] layout; batch 2bp rows on
    # partitions 0:64, batch 2bp+1 rows on 64:128 (index permutation only).
    x16 = np.asarray(x, dtype=np.float32).astype(np.float16)
    xv = x16.reshape(N_CORES, N_BPAIR, 2, C, H, W)
    xP = np.empty((N_CORES, N_BPAIR, 128, C, W), dtype=np.float16)
    xP[:, :, 0:64] = xv[:, :, 0].transpose(0, 1, 3, 2, 4)   # [i,bp,y,c,x]
    xP[:, :, 64:128] = xv[:, :, 1].transpose(0, 1, 3, 2, 4)
    xP = xP.reshape(N_CORES, N_BPAIR, 128, XCOLS)

    in_maps = []
    for i in range(N_CORES):
        in_maps.append({"x": xP[i], "ops1": ops1, "ops2": ops2})
    res = run_bass_kernel_spmd(nc, in_maps, list(range(N_CORES)))
    _LAST_RESULT = res

    # host unpack: y[i] [bp, (c2, x'), (g, m, s, y')];
    # channel = 8g + 2m + c2, batch = 4i + 2bp + s.
    out = np.empty((B_FULL, C, H, W), dtype=np.float32)
    for i in range(N_CORES):
        yv = res.results[i]["y"].astype(np.float32).reshape(
            N_BPAIR, 2, W, P_POS, GROUP // 2, 2, H)
        yt = yv.transpose(0, 5, 3, 4, 1, 6, 2)   # bp s g m c2 y' x'
        out[4 * i:4 * i + 4] = yt.reshape(B_LOC, C, H, W)
    return out


if __name__ == "__main__":
    nc = _build_bass()
    print("bass program built ok")


# revision 10
# speedup vs baseline: 1.1324x; 1.0795x over previous
"""DisplaceChannel (integer displace + per-position 5x5 gaussian depthwise
conv) as a Bass/Tile kernel for 8 Trainium2 NeuronCores — v8.

Math: separable gaussian; integer shift + 'same' zero-padding fold into
banded 64x64 row/col operators built host-side.  Per image:
out = R1^T X R2.

Per 2-group block (2 positions x 8 channels x one batch-pair), PE work:
  pass1 (image-stationary, transposing): 8x MM N=128, lhsT = wt[:, 128]
        (2 channels, both batches), rhs = blockdiag(R1,R1)
  pass2 (operator-stationary): 2x MM N=512, lhsT = blockdiag(R2,R2),
        rhs = pass1 result (fused fp16 copy [128,1024])
PSUM evacuation is fused across the 2 groups ([128,1024] copies) to
amortize per-op overhead on Vector/Scalar.

DMA rings: input on sync HWDGE (pure stream, nothing blocking it),
operator consts on scalar HWDGE (parallel FIFO), outputs on the
otherwise-idle GpSimd SWDGE ring.  Input chunks ramp 2->12 groups.

Sharding: data-parallel over batch (4 per core); operators replicated.
"""

import numpy as np

from concourse import bacc, mybir, tile
from concourse.bass_utils import run_bass_kernel_spmd

# problem constants (hardcoded per harness contract)
B_FULL, C, H, W = 32, 384, 64, 64
N_CORES = 8
B_LOC = B_FULL // N_CORES          # 4 batches per core
P_POS = 48                         # offset positions; C // P_POS = 8 chan/pos
GROUP = C // P_POS                 # 8 channels share one operator pair
KSZ, SIGMA, CK = 5, 0.5, 2

N_BPAIR = B_LOC // 2               # batch-pairs (2bp, 2bp+1) per core
XCOLS = C * 64                     # 24576 per-bp packed cols
GCOLS = GROUP * 64                 # 512 cols per group

FP16 = mybir.dt.float16
FP32 = mybir.dt.float32

_LAST_RESULT = None                # test.py introspection (profile/exec time)

# per-bp input chunk schedule (group-start offsets), output chunks, and
# operator-constant slices (position ranges)
IN_STARTS = {0: [0, 2, 4, 8, 12, 24, 36], 1: [0, 12, 24, 36]}
OUT_CHUNKS = [(0, 4), (4, 12), (12, 24), (24, 36), (36, 44), (44, 48)]
OPS_SLICES = [(0, 8), (8, 24), (24, 48)]


def _shift_conv_matrix(sub, d):
    """[64(src), 64(out)] with R[src,out] = k[i], src = out + i - 2 - d,
    masked by conv zero-pad (0<=out+i-2<64) and shift zero-fill (0<=src<64)."""
    k = np.exp(-((np.arange(KSZ) - CK + sub) ** 2) / (2.0 * SIGMA**2))
    k = k / k.sum()
    R = np.zeros((H, H), dtype=np.float64)
    out = np.arange(H)
    for i in range(KSZ):
        t = out + i - CK            # coordinate in the shifted image
        src = t - d
        m = (t >= 0) & (t < H) & (src >= 0) & (src < H)
        R[src[m], out[m]] += k[i]
    return R


def _build_ops(offset):
    """ops1 [128, 48*128] fp16 : per position blockdiag(R1, R1)
    ops2 [128, 48*128] fp16 : per position blockdiag(R2, R2)"""
    off_round = np.round(offset.astype(np.float64))
    off_int = off_round.astype(np.int64)
    sub = offset.astype(np.float64) - off_round
    ops1 = np.zeros((128, P_POS * 128), dtype=np.float64)
    ops2 = np.zeros((128, P_POS * 128), dtype=np.float64)
    for p in range(P_POS):
        R1 = _shift_conv_matrix(sub[p, 1], off_int[p, 1])   # y: suby, dy
        R2 = _shift_conv_matrix(sub[p, 0], off_int[p, 0])   # x: subx, dx
        ops1[0:64, 128 * p:128 * p + 64] = R1
        ops1[64:128, 128 * p + 64:128 * p + 128] = R1
        ops2[0:64, 128 * p:128 * p + 64] = R2
        ops2[64:128, 128 * p + 64:128 * p + 128] = R2
    return ops1.astype(np.float16), ops2.astype(np.float16)


def _build_bass():
    nc = bacc.Bacc(
        "TRN2",
        target_bir_lowering=False,
        debug=False,
        num_devices=N_CORES,
    )
    # packed fp16 input: per bp a [128, 24576] block; channel c at cols
    # 64c:64c+64, batch 2bp rows on partitions 0:64, batch 2bp+1 on 64:128.
    x_in = nc.declare_dram_parameter("x", [N_BPAIR, 128, XCOLS], FP16,
                                     isOutput=False)
    ops1_in = nc.declare_dram_parameter("ops1", [128, P_POS * 128], FP16,
                                        isOutput=False)
    ops2_in = nc.declare_dram_parameter("ops2", [128, P_POS * 128], FP16,
                                        isOutput=False)
    # packed output: per bp [128, 24576] fp16 (host upcasts to f32);
    # partitions (c2, x'), cols (g, m, s, y'); channel = 8g + 2m + c2,
    # batch = 2bp + s.
    y_out = nc.declare_dram_parameter("y", [N_BPAIR, 128, XCOLS], FP16,
                                      isOutput=True)

    with tile.TileContext(nc) as tc:
        with (
            tc.tile_pool(name="consts", bufs=1) as consts,
            tc.tile_pool(name="wsmall", bufs=2) as wpool,
            tc.tile_pool(name="w12", bufs=6) as wpool12,
            tc.tile_pool(name="l2", bufs=4) as l2pool,
            tc.tile_pool(name="outs", bufs=5) as outpool,
            tc.tile_pool(name="psum1", bufs=3, space="PSUM") as psum1p,
            tc.tile_pool(name="psum2", bufs=3, space="PSUM") as psum2p,
        ):
            # operator constants, sliced so early groups unblock fast;
            # first (tiny) slice on the sync ring ahead of the input
            # stream, the rest on the scalar HWDGE ring in parallel.
            t_ops1, t_ops2 = {}, {}
            for (a, b) in OPS_SLICES:
                t_ops1[a] = consts.tile([128, (b - a) * 128], FP16,
                                        tag=f"ops1_{a}", name=f"t_ops1_{a}")
                t_ops2[a] = consts.tile([128, (b - a) * 128], FP16,
                                        tag=f"ops2_{a}", name=f"t_ops2_{a}")

            def ops_slice(t, g):
                for (a, b) in OPS_SLICES:
                    if a <= g < b:
                        return t[a][:, 128 * (g - a):128 * (g - a) + 128]
                raise AssertionError(g)

            for (a, b) in OPS_SLICES:
                nc.scalar.dma_start(out=t_ops1[a][:],
                                    in_=ops1_in[:, 128 * a:128 * b])
                nc.scalar.dma_start(out=t_ops2[a][:],
                                    in_=ops2_in[:, 128 * a:128 * b])

            # input chunk bookkeeping (sync HWDGE ring, emission order =
            # trigger order; pool bufs throttle prefetch depth)
            chunk_list = []                     # (bp, ga, gb)
            for bp in range(N_BPAIR):
                starts = IN_STARTS[bp]
                for ci, a in enumerate(starts):
                    b = starts[ci + 1] if ci + 1 < len(starts) else P_POS
                    chunk_list.append((bp, a, b))
            g2chunk = {}
            for ci, (bp, a, b) in enumerate(chunk_list):
                for g in range(a, b):
                    g2chunk[(bp, g)] = ci
            wts = {}
            n_emitted = 0

            def emit_in_dma(ci):
                bp, a, b = chunk_list[ci]
                gsz = b - a
                pool = wpool12 if gsz == 12 else wpool
                wt = pool.tile([128, gsz * GCOLS], FP16, tag=f"w{gsz}")
                nc.sync.dma_start(
                    out=wt[:],
                    in_=x_in[bp][:, a * GCOLS:b * GCOLS])
                wts[ci] = wt

            LOOKAHEAD = 3
            it = 0
            outs = None
            oc = None
            for bp in range(N_BPAIR):
                for g in range(P_POS):                  # 48 groups / bp
                    ci = g2chunk[(bp, g)]
                    while n_emitted <= min(ci + LOOKAHEAD, len(chunk_list) - 1):
                        emit_in_dma(n_emitted)
                        n_emitted += 1
                    for s_, e_ in OUT_CHUNKS:
                        if g == s_:
                            outs = outpool.tile([128, (e_ - s_) * GCOLS],
                                                FP16, tag="outs")
                            oc = (s_, e_)
                    wt = wts[ci]
                    ca = chunk_list[ci][1]          # chunk's first group

                    ps1 = psum1p.tile([128, 512], FP32)
                    r1 = ops_slice(t_ops1, g)
                    for m in range(GROUP // 2):          # 4 channel-pairs
                        cs = slice((g - ca) * GCOLS + 128 * m,
                                   (g - ca) * GCOLS + 128 * m + 128)
                        nc.tensor.matmul(ps1[:, 128 * m:128 * m + 128],
                                         wt[:, cs], r1,
                                         start=True, stop=True)
                    # psum fp32 -> sbuf fp16 (pass2 moving operand)
                    l2 = l2pool.tile([128, 512], FP16)
                    if it % 2 == 0:
                        nc.vector.tensor_copy(l2[:], ps1[:])
                    else:
                        nc.scalar.copy(l2[:], ps1[:])
                    ps2 = psum2p.tile([128, 512], FP32)
                    nc.tensor.matmul(ps2[:], ops_slice(t_ops2, g), l2[:],
                                     start=True, stop=True)
                    # final psum -> staging (fp16)
                    od = outs[:, 512 * (g - oc[0]):512 * (g - oc[0]) + 512]
                    if it % 2 == 0:
                        nc.scalar.copy(od, ps2[:])
                    else:
                        nc.vector.tensor_copy(od, ps2[:])
                    it += 1

                    if g + 1 == oc[1]:
                        nc.gpsimd.dma_start(
                            out=y_out[bp][:, GCOLS * oc[0]:GCOLS * oc[1]],
                            in_=outs[:])
    nc.compile()
    return nc


_NC_CACHE = None


def kernel(x: np.ndarray, offset: np.ndarray) -> np.ndarray:
    global _LAST_RESULT, _NC_CACHE
    assert x.shape == (B_FULL, C, H, W), x.shape
    ops1, ops2 = _build_ops(np.asarray(offset, dtype=np.float32))
    if _NC_CACHE is None:
        _NC_CACHE = _build_bass()
    nc = _NC_CACHE

    # host pack: fp16 cast + [p, (c, x)] layout; batch 2bp rows on
    # partitions 0:64, batch 2bp+1 rows on 64:128 (index permutation only).
    x16 = np.asarray(x, dtype=np.float32).astype(np.float16)
    xv = x16.reshape(N_CORES, N_BPAIR, 2, C, H, W)
    xP = np.empty((N_CORES, N_BPAIR, 128, C, W), dtype=np.float16)
    xP[:, :, 0:64] = xv[:, :, 0].transpose(0, 1, 3, 2, 4)   # [i,bp,y,c,x]
    xP[:, :, 64:128] = xv[:, :, 1].transpose(0, 1, 3, 2, 4)
    xP = xP.reshape(N_CORES, N_BPAIR, 128, XCOLS)

    in_maps = []
    for i in range(N_CORES):
        in_maps.append({"x": xP[i], "ops1": ops1, "ops2": ops2})
    res = run_bass_kernel_spmd(nc, in_maps, list(range(N_CORES)))
    _LAST_RESULT = res

    # host unpack: y[i] [bp, (c2, x'), (g, m, s, y')];
    # channel = 8g + 2m + c2, batch = 4i + 2bp + s.
    out = np.empty((B_FULL, C, H, W), dtype=np.float32)
    for i in range(N_CORES):
        yv = res.results[i]["y"].astype(np.float32).reshape(
            N_BPAIR, 2, W, P_POS, GROUP // 2, 2, H)
        yt = yv.transpose(0, 5, 3, 4, 1, 6, 2)   # bp s g m c2 y' x'
        out[4 * i:4 * i + 4] = yt.reshape(B_LOC, C, H, W)
    return out


if __name__ == "__main__":
    nc = _build_bass()
    print("bass program built ok")
